# revision 30
# baseline (speedup 1.0000x reference)
"""Trainium2 Bass kernel for nn_AttentionZP (swishmax attention, B=4 Q=1024 K=1024
T=512 H=8 A=64 C=128), SPMD across 8 NeuronCores.

Sharding: core c handles batch b = c//2 and heads [4*(c%2), 4*(c%2)+4).
Each core computes a [T, Q] partial output (sum over its 4 heads); the host sums
the two partials per batch and transposes to [Q, T].

Math (per batch b, head h), exactly reassociated from the reference:
  kT[a,k]   = sum_t key_tokens[b,k,t]  * key_down[h,t,a]           (f32 matmul)
  qT[a,q]   = sum_t query_tokens[b,q,t]* query_down[h,t,a] + bias  (f32 matmul)
  x[q,k]    = sum_a kT[a,k]*qT[a,q]                                (f32 matmul, [Q,K] layout)
  negM[q]   = -max_k x[q,k]                (DVE reduce_max negate=True, from PSUM)
  e[q,k]    = exp(x + negM)                (ACT from PSUM f32, per-partition bias)
  xe[q,k]   = x*e                          (DVE tensor_tensor, bf16 out)
  KC[k,c]   = sum_t key_tokens[b,k,t]*value_down[h,t,c]            (bf16 matmul)
  VSc[q,c+1]= sum_k xe[k,q]*[KC|ones][k,c] (xe DMA-xbar-transposed to [K,Q] first;
                                            col 128 gives T[q] = sum_k xe)
  VScN[q,c] = VSc[q,c]/T[q]                (ACT copyback with per-partition scale;
              dist = xe/T; the reference's "+1" is dropped: T ~ |x_max| ~ 1.5e4,
              so the error is ~1e-4 relative)
  out[t,q] += sum_c value_up[h,c,t]*VScN_T[c,q]  (PSUM-accumulated over h)
"""

import os
import sys

sys.path.insert(0, "/opt/trn_rl_repo")

import numpy as np
import ml_dtypes

BF16NP = ml_dtypes.bfloat16

_NC = None


def _build_nc():
    import concourse.bass as bass
    import concourse.tile as tile
    import concourse.mybir as mybir
    from concourse import bacc
    from concourse.bass import ds, ts

    F32 = mybir.dt.float32
    BF16 = mybir.dt.bfloat16
    AF = mybir.ActivationFunctionType
    OP = mybir.AluOpType
    AX = mybir.AxisListType

    nc = bacc.Bacc()

    ktokTb = nc.dram_tensor("ktokTb", [512, 1024], BF16, kind="ExternalInput")
    ktokTl = nc.dram_tensor("ktokTl", [512, 1024], BF16, kind="ExternalInput")
    qtokTb = nc.dram_tensor("qtokTb", [512, 1024], BF16, kind="ExternalInput")
    qtokTl = nc.dram_tensor("qtokTl", [512, 1024], BF16, kind="ExternalInput")
    kd = nc.dram_tensor("kd", [512, 2, 256], BF16, kind="ExternalInput")
    qd = nc.dram_tensor("qd", [512, 2, 256], BF16, kind="ExternalInput")
    qdb = nc.dram_tensor("qdb", [128, 2], F32, kind="ExternalInput")
    vd = nc.dram_tensor("vd", [512, 512], BF16, kind="ExternalInput")
    vu = nc.dram_tensor("vu", [128, 4, 512], BF16, kind="ExternalInput")
    out = nc.dram_tensor("out", [512, 1024], F32, kind="ExternalOutput")

    STAGE = int(os.environ.get("KSTAGE", "99"))

    with tile.TileContext(nc) as tc:
        with (
            tc.tile_pool(name="singles", bufs=1) as singles,
            tc.tile_pool(name="lgps", bufs=3, space="PSUM") as lgps,
            tc.tile_pool(name="mmps", bufs=2, space="PSUM") as mmps,
            tc.tile_pool(name="eexp", bufs=3) as eexpp,
            tc.tile_pool(name="xexp", bufs=3) as xexpp,
            tc.tile_pool(name="hsml", bufs=2) as hsml,
            tc.tile_pool(name="obuf", bufs=3) as obuf,
        ):
            # ---- persistent SBUF tensors -----------------------------------
            ktokTb_sb = singles.tile([128, 4, 1024], BF16)
            nc.scalar.dma_start(ktokTb_sb[:], ktokTb[:].rearrange("(a p) k -> p a k", p=128))
            vd_sb = singles.tile([128, 4, 512], BF16)
            nc.scalar.dma_start(vd_sb[:], vd[:].rearrange("(a p) m -> p a m", p=128))
            kd_sb = singles.tile([128, 4, 2, 256], BF16)
            nc.scalar.dma_start(kd_sb[:], kd[:].rearrange("(a p) two m -> p a two m", p=128))
            ktokTl_sb = singles.tile([128, 4, 1024], BF16)
            nc.scalar.dma_start(ktokTl_sb[:], ktokTl[:].rearrange("(a p) k -> p a k", p=128))
            qd_sb = singles.tile([128, 4, 2, 256], BF16)
            nc.scalar.dma_start(qd_sb[:], qd[:].rearrange("(a p) two m -> p a two m", p=128))
            qdb_sb = singles.tile([128, 2], F32)
            nc.scalar.dma_start(qdb_sb[:], qdb[:])
            qtokTb_sb = singles.tile([128, 4, 1024], BF16)
            nc.scalar.dma_start(qtokTb_sb[:], qtokTb[:].rearrange("(a p) k -> p a k", p=128))
            qtokTl_sb = singles.tile([128, 4, 1024], BF16)
            nc.scalar.dma_start(qtokTl_sb[:], qtokTl[:].rearrange("(a p) k -> p a k", p=128))
            vu_sb = singles.tile([128, 4, 512], BF16)
            nc.scalar.dma_start(vu_sb[:], vu[:])

            # KC with a ones column per head: [k-part, kc, h, 0:128]=KC, [...,128]=1
            KC_sb = singles.tile([128, 8, 4, 132], BF16)
            nc.vector.memset(KC_sb[:, :, :, 128:132], 1.0)
            kT_sb = singles.tile([128, 2, 2, 1024], BF16)  # [part, g, hi/lo, k]
            qT_sb = singles.tile([128, 2, 2, 1024], BF16)
            xexpT_sb = [
                [singles.tile([128, 8, 1024], BF16, name=f"xexpT{gg}{i}") for i in range(2)]
                for gg in range(2)
            ]
            VScN_sb = singles.tile([128, 4, 8, 128], BF16)
            VScNT_sb = singles.tile([128, 4, 1024], BF16)
            if STAGE < 7:
                nc.vector.memset(VScNT_sb[:], 0.0)
            if STAGE < 5:
                for gg in range(2):
                    nc.vector.memset(xexpT_sb[gg][0][:], 0.0)
                    nc.vector.memset(xexpT_sb[gg][1][:], 0.0)

            # ---- phase 0: PE warmup during the input-DMA wait --------------
            # HAM releases the 1.2->2.4 GHz clock gate after ~3.4us of dense
            # array activity; the PE is idle anyway while inputs stream in.
            # Scratch results go to an lg-pool PSUM tile (idle until phase 3)
            # so phase 1/2's mm-pool slots are untouched.
            wsc = singles.tile([128, 640], BF16)
            nc.vector.memset(wsc[:], 0.0)
            wps = lgps.tile([128, 1024], F32, tag="lg", name="warm")
            for w in range(16):
                nc.tensor.matmul(
                    wps[:, ts(w % 2, 512)], wsc[:, 0:128], wsc[:, 128:640],
                    start=True, stop=True,
                )

            # ---- phase 1: KC[k, c] for all 4 heads -------------------------
            for kc in range(8):
                ps = mmps.tile([128, 512], F32, tag="mm")
                for t in range(4):
                    nc.tensor.matmul(
                        ps[:], ktokTb_sb[:, t, ts(kc, 128)], vd_sb[:, t, :],
                        start=(t == 0), stop=(t == 3),
                    )
                nc.scalar.copy(
                    KC_sb[:, kc, :, 0:128], ps[:].rearrange("p (h c) -> p h c", c=128)
                )

            # ---- phase 2: kT / qT projections (2 head-pairs packed) --------
            for g in range(2):
                for kh in range(2):
                    ps = mmps.tile([128, 512], F32, tag="mm")
                    first = True
                    for t in range(4):
                        for wsel, asel in ((0, ktokTb_sb), (0, ktokTl_sb), (1, ktokTb_sb)):
                            nc.tensor.matmul(
                                ps[:], kd_sb[:, t, wsel, ts(g, 128)],
                                asel[:, t, ts(kh, 512)],
                                start=first, stop=(t == 3 and wsel == 1),
                            )
                            first = False
                    nc.scalar.copy(kT_sb[:, g, 0, ts(kh, 512)], ps[:])
                    nc.vector.tensor_tensor(
                        kT_sb[:, g, 1, ts(kh, 512)], ps[:],
                        kT_sb[:, g, 0, ts(kh, 512)], OP.subtract,
                    )
                for qh in range(2):
                    ps = mmps.tile([128, 512], F32, tag="mm")
                    first = True
                    for t in range(4):
                        for wsel, asel in ((0, qtokTb_sb), (0, qtokTl_sb), (1, qtokTb_sb)):
                            nc.tensor.matmul(
                                ps[:], qd_sb[:, t, wsel, ts(g, 128)],
                                asel[:, t, ts(qh, 512)],
                                start=first, stop=(t == 3 and wsel == 1),
                            )
                            first = False
                    nc.scalar.activation(
                        qT_sb[:, g, 0, ts(qh, 512)], ps[:], AF.Identity,
                        bias=qdb_sb[:, g : g + 1], scale=1.0,
                    )
                    nc.vector.scalar_tensor_tensor(
                        out=qT_sb[:, g, 1, ts(qh, 512)], in0=ps[:],
                        scalar=qdb_sb[:, g : g + 1], in1=qT_sb[:, g, 0, ts(qh, 512)],
                        op0=OP.add, op1=OP.subtract,
                    )

            # ---- phase 3: per head-PAIR: interleaved logits (row-group
            # concurrency), swishmax, transpose, VSc ------------------------
            def do_vsc(g, hh, qc, recipFs_g, xexpT_g):
                h = 2 * g + hh
                vps = mmps.tile([128, 512], F32, tag="mm", name=f"vps{g}{hh}{qc}")
                for kc in range(8):
                    nc.tensor.matmul(
                        vps[:, 0:129],
                        xexpT_g[hh][:, kc, ts(qc, 128)],
                        KC_sb[:, kc, h, 0:129],
                        start=(kc == 0), stop=(kc == 7),
                    )
                nc.vector.reciprocal(recipFs_g[hh][:, qc : qc + 1], vps[:, 128:129])
                nc.scalar.activation(
                    VScN_sb[:, h, qc, :], vps[:, 0:128], AF.Identity,
                    bias=0.0, scale=recipFs_g[hh][:, qc : qc + 1],
                )
                if qc == 7:
                    nc.sync.dma_start_transpose(
                        VScNT_sb[:, h, :].rearrange("p (a b) -> p a b", b=128),
                        VScN_sb[:, h, :, :],
                    )

            for g in range(2):
                negMs = [hsml.tile([128, 8], F32, tag=f"negM{i}", name=f"negM{g}{i}") for i in range(2)]
                recipFs = [hsml.tile([128, 8], F32, tag=f"recipF{i}", name=f"recipF{g}{i}") for i in range(2)]
                for qc in range(8):
                    lgs = [lgps.tile([128, 1024], F32, tag="lg", name=f"lg{g}{qc}{i}") for i in range(2)]
                    for nh in range(2):
                        for wq, wk, fst, lst in ((0, 0, True, False), (0, 1, False, False), (1, 0, False, True)):
                            for hh in range(2):
                                off = 64 * hh
                                nc.tensor.matmul(
                                    lgs[hh][:, ts(nh, 512)],
                                    qT_sb[ds(off, 64), g, wq, ts(qc, 128)],
                                    kT_sb[ds(off, 64), g, wk, ts(nh, 512)],
                                    start=fst, stop=lst,
                                    tile_position=(off, 0),
                                )
                    for hh in range(2):
                        lg = lgs[hh]
                        xe = xexpp.tile([128, 1024], BF16, tag="xexp")
                        if STAGE >= 3:
                            nc.vector.reduce_max(
                                negMs[hh][:, qc : qc + 1], lg[:], axis=AX.X, negate=True,
                            )
                        if STAGE >= 4:
                            ee = eexpp.tile([128, 1024], BF16, tag="eexp")
                            nc.scalar.activation(
                                ee[:], lg[:], AF.Exp, bias=negMs[hh][:, qc : qc + 1], scale=1.0,
                            )
                            nc.vector.tensor_tensor(xe[:], lg[:], ee[:], OP.mult)
                        else:
                            nc.vector.tensor_copy(xe[:], lg[:])
                        # batched xbar transpose: xexpT[p, kc, qc*128+j] = xe[j, kc*128+p]
                        if STAGE >= 5:
                            nc.sync.dma_start_transpose(
                                xexpT_sb[g][hh][:, :, ts(qc, 128)], xe[:]
                            )
                        else:
                            nc.scalar.dma_start(
                                out[ds(0, 128), ds(0, 512)], xe[:, 0:1024].bitcast(F32)
                            )
                    if STAGE >= 7 and qc > 0:
                        for hh in range(2):
                            do_vsc(g, hh, qc - 1, recipFs, xexpT_sb[g])
                if STAGE >= 7:
                    for hh in range(2):
                        do_vsc(g, hh, 7, recipFs, xexpT_sb[g])

            # ---- phase 4: VST[t,q] = sum_h vu[h]^T @ VScNT[h] --------------
            for t_ in range(4):
                for qh in range(2):
                    vps = mmps.tile([128, 512], F32, tag="mm")
                    for h in range(4):
                        nc.tensor.matmul(
                            vps[:], vu_sb[:, h, ts(t_, 128)], VScNT_sb[:, h, ts(qh, 512)],
                            start=(h == 0), stop=(h == 3),
                        )
                    ob = obuf.tile([128, 512], F32, tag="ob")
                    if (t_ * 2 + qh) % 2 == 0:
                        nc.scalar.copy(ob[:], vps[:])
                        nc.scalar.dma_start(out[ds(t_ * 128, 128), ds(qh * 512, 512)], ob[:])
                    else:
                        nc.vector.tensor_copy(ob[:], vps[:])
                        nc.sync.dma_start(out[ds(t_ * 128, 128), ds(qh * 512, 512)], ob[:])

    nc.compile()
    return nc


def _get_nc():
    global _NC
    if _NC is None:
        _NC = _build_nc()
    return _NC


def _make_in_maps(inputs):
    kt = np.asarray(inputs["key_tokens"], dtype=np.float32)
    qt = np.asarray(inputs["query_tokens"], dtype=np.float32)
    kdw = np.asarray(inputs["key_down"], dtype=np.float32)
    qdw = np.asarray(inputs["query_down"], dtype=np.float32)
    qdbw = np.asarray(inputs["query_down_bias"], dtype=np.float32)
    vdw = np.asarray(inputs["value_down"], dtype=np.float32)
    vuw = np.asarray(inputs["value_up"], dtype=np.float32)

    in_maps = []
    for c in range(8):
        b, g2 = c // 2, c % 2
        hs = [4 * g2 + j for j in range(4)]
        ktokT = np.ascontiguousarray(kt[b].T)
        qtokT = np.ascontiguousarray(qt[b].T)
        ktokThi = ktokT.astype(BF16NP)
        ktokTlo = (ktokT - ktokThi.astype(np.float32)).astype(BF16NP)
        qtokThi = qtokT.astype(BF16NP)
        qtokTlo = (qtokT - qtokThi.astype(np.float32)).astype(BF16NP)
        kdp = np.ascontiguousarray(np.concatenate([kdw[h] for h in hs], axis=1))
        qdp = np.ascontiguousarray(np.concatenate([qdw[h] for h in hs], axis=1))
        kdhi = kdp.astype(BF16NP)
        kdlo = (kdp - kdhi.astype(np.float32)).astype(BF16NP)
        qdhi = qdp.astype(BF16NP)
        qdlo = (qdp - qdhi.astype(np.float32)).astype(BF16NP)
        qdbp = np.stack(
            [
                np.concatenate([qdbw[hs[0]][0], qdbw[hs[1]][0]]),
                np.concatenate([qdbw[hs[2]][0], qdbw[hs[3]][0]]),
            ],
            axis=1,
        ).astype(np.float32)
        vdp = np.ascontiguousarray(np.concatenate([vdw[h] for h in hs], axis=1))
        vup = np.ascontiguousarray(np.transpose(vuw[hs], (1, 0, 2)))
        in_maps.append(
            {
                "ktokTb": ktokThi,
                "ktokTl": ktokTlo,
                "qtokTb": qtokThi,
                "qtokTl": qtokTlo,
                "kd": np.ascontiguousarray(np.stack([kdhi, kdlo], axis=1)),
                "qd": np.ascontiguousarray(np.stack([qdhi, qdlo], axis=1)),
                "qdb": qdbp,
                "vd": vdp.astype(BF16NP),
                "vu": vup.astype(BF16NP),
            }
        )
    return in_maps


def _ensure_ntff_hook():
    """The agent image's antenv lacks axon_hooks; shim it so trace=True works."""
    import types

    if "antenv.axon_hooks" in sys.modules:
        return
    import antenv

    mod = types.ModuleType("antenv.axon_hooks")
    _hook = [None]
    mod.set_axon_ntff_profile_hook = lambda h: _hook.__setitem__(0, h)
    mod.get_axon_ntff_profile_hook = lambda: _hook[0]
    sys.modules["antenv.axon_hooks"] = mod
    antenv.axon_hooks = mod
    try:
        from trn_agent_boot.trn_boot import _ntff_profile_via_ctypes

        mod.set_axon_ntff_profile_hook(
            _ntff_profile_via_ctypes("/opt/axon/libaxon_pjrt.so")
        )
    except Exception:
        pass


def run(inputs, trace=False):
    """Run the SPMD kernel; returns (output [4,1024,512] f32, BassKernelResults)."""
    if trace:
        _ensure_ntff_hook()
    from concourse.bass_utils import run_bass_kernel_spmd

    nc = _get_nc()
    in_maps = _make_in_maps(inputs)
    res = run_bass_kernel_spmd(nc, in_maps, core_ids=list(range(8)), trace=trace)
    outs = []
    for b in range(4):
        part = res.results[2 * b]["out"] + res.results[2 * b + 1]["out"]
        outs.append(np.ascontiguousarray(part.T))
    return np.stack(outs).astype(np.float32), res


def kernel(**inputs) -> np.ndarray:
    out, _ = run(inputs, trace=False)
    return out


# revision 31
# speedup vs baseline: 1.0427x; 1.0427x over previous
"""Trainium2 Bass kernel for nn_AttentionZP (swishmax attention, B=4 Q=1024 K=1024
T=512 H=8 A=64 C=128), SPMD across 8 NeuronCores.

Sharding: core c handles batch b = c//2 and heads [4*(c%2), 4*(c%2)+4).
Each core computes a [T, Q] partial output (sum over its 4 heads); the host sums
the two partials per batch and transposes to [Q, T].

Math (per batch b, head h), exactly reassociated from the reference:
  kT[a,k]   = sum_t key_tokens[b,k,t]  * key_down[h,t,a]           (f32 matmul)
  qT[a,q]   = sum_t query_tokens[b,q,t]* query_down[h,t,a] + bias  (f32 matmul)
  x[q,k]    = sum_a kT[a,k]*qT[a,q]                                (f32 matmul, [Q,K] layout)
  negM[q]   = -max_k x[q,k]                (DVE reduce_max negate=True, from PSUM)
  e[q,k]    = exp(x + negM)                (ACT from PSUM f32, per-partition bias)
  xe[q,k]   = x*e                          (DVE tensor_tensor, bf16 out)
  KC[k,c]   = sum_t key_tokens[b,k,t]*value_down[h,t,c]            (bf16 matmul)
  VSc[q,c+1]= sum_k xe[k,q]*[KC|ones][k,c] (xe DMA-xbar-transposed to [K,Q] first;
                                            col 128 gives T[q] = sum_k xe)
  VScN[q,c] = VSc[q,c]/T[q]                (ACT copyback with per-partition scale;
              dist = xe/T; the reference's "+1" is dropped: T ~ |x_max| ~ 1.5e4,
              so the error is ~1e-4 relative)
  out[t,q] += sum_c value_up[h,c,t]*VScN_T[c,q]  (PSUM-accumulated over h)
"""

import os
import sys

sys.path.insert(0, "/opt/trn_rl_repo")

import numpy as np
import ml_dtypes

BF16NP = ml_dtypes.bfloat16

_NC = None


def _build_nc():
    import concourse.bass as bass
    import concourse.tile as tile
    import concourse.mybir as mybir
    from concourse import bacc
    from concourse.bass import ds, ts

    F32 = mybir.dt.float32
    BF16 = mybir.dt.bfloat16
    AF = mybir.ActivationFunctionType
    OP = mybir.AluOpType
    AX = mybir.AxisListType

    nc = bacc.Bacc()

    ktokTb = nc.dram_tensor("ktokTb", [512, 1024], BF16, kind="ExternalInput")
    ktokTl = nc.dram_tensor("ktokTl", [512, 1024], BF16, kind="ExternalInput")
    qtokTb = nc.dram_tensor("qtokTb", [512, 1024], BF16, kind="ExternalInput")
    qtokTl = nc.dram_tensor("qtokTl", [512, 1024], BF16, kind="ExternalInput")
    kd = nc.dram_tensor("kd", [512, 2, 256], BF16, kind="ExternalInput")
    qd = nc.dram_tensor("qd", [512, 2, 256], BF16, kind="ExternalInput")
    qdb = nc.dram_tensor("qdb", [128, 2], F32, kind="ExternalInput")
    vd = nc.dram_tensor("vd", [512, 512], BF16, kind="ExternalInput")
    vu = nc.dram_tensor("vu", [128, 4, 512], BF16, kind="ExternalInput")
    out = nc.dram_tensor("out", [512, 1024], F32, kind="ExternalOutput")

    STAGE = int(os.environ.get("KSTAGE", "99"))

    with tile.TileContext(nc) as tc:
        with (
            tc.tile_pool(name="singles", bufs=1) as singles,
            tc.tile_pool(name="lgps", bufs=3, space="PSUM") as lgps,
            tc.tile_pool(name="mmps", bufs=2, space="PSUM") as mmps,
            tc.tile_pool(name="eexp", bufs=3) as eexpp,
            tc.tile_pool(name="xexp", bufs=3) as xexpp,
            tc.tile_pool(name="hsml", bufs=2) as hsml,
            tc.tile_pool(name="obuf", bufs=3) as obuf,
        ):
            # ---- persistent SBUF tensors -----------------------------------
            ktokTb_sb = singles.tile([128, 4, 1024], BF16)
            nc.scalar.dma_start(ktokTb_sb[:], ktokTb[:].rearrange("(a p) k -> p a k", p=128))
            vd_sb = singles.tile([128, 4, 512], BF16)
            nc.scalar.dma_start(vd_sb[:], vd[:].rearrange("(a p) m -> p a m", p=128))
            kd_sb = singles.tile([128, 4, 2, 256], BF16)
            nc.scalar.dma_start(kd_sb[:], kd[:].rearrange("(a p) two m -> p a two m", p=128))
            ktokTl_sb = singles.tile([128, 4, 1024], BF16)
            nc.scalar.dma_start(ktokTl_sb[:], ktokTl[:].rearrange("(a p) k -> p a k", p=128))
            qd_sb = singles.tile([128, 4, 2, 256], BF16)
            nc.scalar.dma_start(qd_sb[:], qd[:].rearrange("(a p) two m -> p a two m", p=128))
            qdb_sb = singles.tile([128, 2], F32)
            nc.scalar.dma_start(qdb_sb[:], qdb[:])
            qtokTb_sb = singles.tile([128, 4, 1024], BF16)
            nc.scalar.dma_start(qtokTb_sb[:], qtokTb[:].rearrange("(a p) k -> p a k", p=128))
            qtokTl_sb = singles.tile([128, 4, 1024], BF16)
            nc.scalar.dma_start(qtokTl_sb[:], qtokTl[:].rearrange("(a p) k -> p a k", p=128))
            vu_sb = singles.tile([128, 4, 512], BF16)
            nc.scalar.dma_start(vu_sb[:], vu[:])

            # KC with a ones column per head: [k-part, kc, h, 0:128]=KC, [...,128]=1
            KC_sb = singles.tile([128, 8, 4, 132], BF16)
            nc.vector.memset(KC_sb[:, :, :, 128:132], 1.0)
            kT_sb = singles.tile([128, 2, 2, 1024], BF16)  # [part, g, hi/lo, k]
            qT_sb = singles.tile([128, 2, 2, 1024], BF16)
            xexpT_sb = [
                [singles.tile([128, 8, 1024], BF16, name=f"xexpT{gg}{i}") for i in range(2)]
                for gg in range(2)
            ]
            VScN_sb = singles.tile([128, 4, 8, 128], BF16)
            VScNT_sb = singles.tile([128, 4, 1024], BF16)
            if STAGE < 7:
                nc.vector.memset(VScNT_sb[:], 0.0)
            if STAGE < 5:
                for gg in range(2):
                    nc.vector.memset(xexpT_sb[gg][0][:], 0.0)
                    nc.vector.memset(xexpT_sb[gg][1][:], 0.0)

            # ---- phase 0: PE warmup during the input-DMA wait --------------
            # HAM releases the 1.2->2.4 GHz clock gate after ~3.4us of dense
            # array activity; the PE is idle anyway while inputs stream in.
            # Scratch results go to an lg-pool PSUM tile (idle until phase 3)
            # so phase 1/2's mm-pool slots are untouched.
            wsc = singles.tile([128, 640], BF16)
            nc.vector.memset(wsc[:], 0.0)
            wps = lgps.tile([128, 1024], F32, tag="lg", name="warm")
            for w in range(16):
                nc.tensor.matmul(
                    wps[:, ts(w % 2, 512)], wsc[:, 0:128], wsc[:, 128:640],
                    start=True, stop=True,
                )

            # ---- phase 1: KC[k, c] for all 4 heads -------------------------
            for kc in range(8):
                ps = mmps.tile([128, 512], F32, tag="mm")
                for t in range(4):
                    nc.tensor.matmul(
                        ps[:], ktokTb_sb[:, t, ts(kc, 128)], vd_sb[:, t, :],
                        start=(t == 0), stop=(t == 3),
                    )
                nc.scalar.copy(
                    KC_sb[:, kc, :, 0:128], ps[:].rearrange("p (h c) -> p h c", c=128)
                )

            # ---- phase 2: kT / qT projections (2 head-pairs packed) --------
            for g in range(2):
                for kh in range(2):
                    ps = mmps.tile([128, 512], F32, tag="mm")
                    first = True
                    for t in range(4):
                        for wsel, asel in ((0, ktokTb_sb), (0, ktokTl_sb), (1, ktokTb_sb)):
                            nc.tensor.matmul(
                                ps[:], kd_sb[:, t, wsel, ts(g, 128)],
                                asel[:, t, ts(kh, 512)],
                                start=first, stop=(t == 3 and wsel == 1),
                            )
                            first = False
                    nc.scalar.copy(kT_sb[:, g, 0, ts(kh, 512)], ps[:])
                    nc.vector.tensor_tensor(
                        kT_sb[:, g, 1, ts(kh, 512)], ps[:],
                        kT_sb[:, g, 0, ts(kh, 512)], OP.subtract,
                    )
                for qh in range(2):
                    ps = mmps.tile([128, 512], F32, tag="mm")
                    first = True
                    for t in range(4):
                        for wsel, asel in ((0, qtokTb_sb), (0, qtokTl_sb), (1, qtokTb_sb)):
                            nc.tensor.matmul(
                                ps[:], qd_sb[:, t, wsel, ts(g, 128)],
                                asel[:, t, ts(qh, 512)],
                                start=first, stop=(t == 3 and wsel == 1),
                            )
                            first = False
                    nc.scalar.activation(
                        qT_sb[:, g, 0, ts(qh, 512)], ps[:], AF.Identity,
                        bias=qdb_sb[:, g : g + 1], scale=1.0,
                    )
                    nc.vector.scalar_tensor_tensor(
                        out=qT_sb[:, g, 1, ts(qh, 512)], in0=ps[:],
                        scalar=qdb_sb[:, g : g + 1], in1=qT_sb[:, g, 0, ts(qh, 512)],
                        op0=OP.add, op1=OP.subtract,
                    )

            # ---- phase 3: per head-PAIR: interleaved logits (row-group
            # concurrency), swishmax, transpose, VSc ------------------------
            def do_vsc(g, hh, qc, recipFs_g, xexpT_g):
                h = 2 * g + hh
                vps = mmps.tile([128, 512], F32, tag="mm", name=f"vps{g}{hh}{qc}")
                for kc in range(8):
                    nc.tensor.matmul(
                        vps[:, 0:129],
                        xexpT_g[hh][:, kc, ts(qc, 128)],
                        KC_sb[:, kc, h, 0:129],
                        start=(kc == 0), stop=(kc == 7),
                    )
                nc.vector.reciprocal(recipFs_g[hh][:, qc : qc + 1], vps[:, 128:129])
                nc.scalar.activation(
                    VScN_sb[:, h, qc, :], vps[:, 0:128], AF.Identity,
                    bias=0.0, scale=recipFs_g[hh][:, qc : qc + 1],
                )
                if qc == 7:
                    nc.sync.dma_start_transpose(
                        VScNT_sb[:, h, :].rearrange("p (a b) -> p a b", b=128),
                        VScN_sb[:, h, :, :],
                    )

            for g in range(2):
                negMs = [hsml.tile([128, 8], F32, tag=f"negM{i}", name=f"negM{g}{i}") for i in range(2)]
                recipFs = [hsml.tile([128, 8], F32, tag=f"recipF{i}", name=f"recipF{g}{i}") for i in range(2)]
                for qc in range(8):
                    lgs = [lgps.tile([128, 1024], F32, tag="lg", name=f"lg{g}{qc}{i}") for i in range(2)]
                    for nh in range(2):
                        for wq, wk, fst, lst in ((0, 0, True, False), (0, 1, False, False), (1, 0, False, True)):
                            for hh in range(2):
                                off = 64 * hh
                                nc.tensor.matmul(
                                    lgs[hh][:, ts(nh, 512)],
                                    qT_sb[ds(off, 64), g, wq, ts(qc, 128)],
                                    kT_sb[ds(off, 64), g, wk, ts(nh, 512)],
                                    start=fst, stop=lst,
                                    tile_position=(off, 0),
                                )
                    for hh in range(2):
                        lg = lgs[hh]
                        xe = xexpp.tile([128, 1024], BF16, tag="xexp")
                        if STAGE >= 3:
                            nc.vector.reduce_max(
                                negMs[hh][:, qc : qc + 1], lg[:], axis=AX.X, negate=True,
                            )
                        if STAGE >= 4:
                            ee = eexpp.tile([128, 1024], BF16, tag="eexp")
                            nc.scalar.activation(
                                ee[:], lg[:], AF.Exp, bias=negMs[hh][:, qc : qc + 1], scale=1.0,
                            )
                            # split the x*e multiply: GpSimd (idle) takes half
                            # via an ACT-produced bf16 copy; DVE keeps half.
                            lgb = eexpp.tile([128, 512], BF16, tag="lgb")
                            nc.scalar.copy(lgb[:], lg[:, 0:512])
                            nc.gpsimd.tensor_tensor(
                                xe[:, 0:512], lgb[:], ee[:, 0:512], OP.mult
                            )
                            nc.vector.tensor_tensor(
                                xe[:, 512:1024], lg[:, 512:1024], ee[:, 512:1024], OP.mult
                            )
                        else:
                            nc.vector.tensor_copy(xe[:], lg[:])
                        # batched xbar transpose: xexpT[p, kc, qc*128+j] = xe[j, kc*128+p]
                        if STAGE >= 5:
                            nc.sync.dma_start_transpose(
                                xexpT_sb[g][hh][:, :, ts(qc, 128)], xe[:]
                            )
                        else:
                            nc.scalar.dma_start(
                                out[ds(0, 128), ds(0, 512)], xe[:, 0:1024].bitcast(F32)
                            )
                    if STAGE >= 7 and qc > 0:
                        for hh in range(2):
                            do_vsc(g, hh, qc - 1, recipFs, xexpT_sb[g])
                if STAGE >= 7:
                    for hh in range(2):
                        do_vsc(g, hh, 7, recipFs, xexpT_sb[g])

            # ---- phase 4: VST[t,q] = sum_h vu[h]^T @ VScNT[h] --------------
            for t_ in range(4):
                for qh in range(2):
                    vps = mmps.tile([128, 512], F32, tag="mm")
                    for h in range(4):
                        nc.tensor.matmul(
                            vps[:], vu_sb[:, h, ts(t_, 128)], VScNT_sb[:, h, ts(qh, 512)],
                            start=(h == 0), stop=(h == 3),
                        )
                    ob = obuf.tile([128, 512], F32, tag="ob")
                    if (t_ * 2 + qh) % 2 == 0:
                        nc.scalar.copy(ob[:], vps[:])
                        nc.scalar.dma_start(out[ds(t_ * 128, 128), ds(qh * 512, 512)], ob[:])
                    else:
                        nc.vector.tensor_copy(ob[:], vps[:])
                        nc.sync.dma_start(out[ds(t_ * 128, 128), ds(qh * 512, 512)], ob[:])

    nc.compile()
    return nc


def _get_nc():
    global _NC
    if _NC is None:
        _NC = _build_nc()
    return _NC


def _make_in_maps(inputs):
    kt = np.asarray(inputs["key_tokens"], dtype=np.float32)
    qt = np.asarray(inputs["query_tokens"], dtype=np.float32)
    kdw = np.asarray(inputs["key_down"], dtype=np.float32)
    qdw = np.asarray(inputs["query_down"], dtype=np.float32)
    qdbw = np.asarray(inputs["query_down_bias"], dtype=np.float32)
    vdw = np.asarray(inputs["value_down"], dtype=np.float32)
    vuw = np.asarray(inputs["value_up"], dtype=np.float32)

    in_maps = []
    for c in range(8):
        b, g2 = c // 2, c % 2
        hs = [4 * g2 + j for j in range(4)]
        ktokT = np.ascontiguousarray(kt[b].T)
        qtokT = np.ascontiguousarray(qt[b].T)
        ktokThi = ktokT.astype(BF16NP)
        ktokTlo = (ktokT - ktokThi.astype(np.float32)).astype(BF16NP)
        qtokThi = qtokT.astype(BF16NP)
        qtokTlo = (qtokT - qtokThi.astype(np.float32)).astype(BF16NP)
        kdp = np.ascontiguousarray(np.concatenate([kdw[h] for h in hs], axis=1))
        qdp = np.ascontiguousarray(np.concatenate([qdw[h] for h in hs], axis=1))
        kdhi = kdp.astype(BF16NP)
        kdlo = (kdp - kdhi.astype(np.float32)).astype(BF16NP)
        qdhi = qdp.astype(BF16NP)
        qdlo = (qdp - qdhi.astype(np.float32)).astype(BF16NP)
        qdbp = np.stack(
            [
                np.concatenate([qdbw[hs[0]][0], qdbw[hs[1]][0]]),
                np.concatenate([qdbw[hs[2]][0], qdbw[hs[3]][0]]),
            ],
            axis=1,
        ).astype(np.float32)
        vdp = np.ascontiguousarray(np.concatenate([vdw[h] for h in hs], axis=1))
        vup = np.ascontiguousarray(np.transpose(vuw[hs], (1, 0, 2)))
        in_maps.append(
            {
                "ktokTb": ktokThi,
                "ktokTl": ktokTlo,
                "qtokTb": qtokThi,
                "qtokTl": qtokTlo,
                "kd": np.ascontiguousarray(np.stack([kdhi, kdlo], axis=1)),
                "qd": np.ascontiguousarray(np.stack([qdhi, qdlo], axis=1)),
                "qdb": qdbp,
                "vd": vdp.astype(BF16NP),
                "vu": vup.astype(BF16NP),
            }
        )
    return in_maps


def _ensure_ntff_hook():
    """The agent image's antenv lacks axon_hooks; shim it so trace=True works."""
    import types

    if "antenv.axon_hooks" in sys.modules:
        return
    import antenv

    mod = types.ModuleType("antenv.axon_hooks")
    _hook = [None]
    mod.set_axon_ntff_profile_hook = lambda h: _hook.__setitem__(0, h)
    mod.get_axon_ntff_profile_hook = lambda: _hook[0]
    sys.modules["antenv.axon_hooks"] = mod
    antenv.axon_hooks = mod
    try:
        from trn_agent_boot.trn_boot import _ntff_profile_via_ctypes

        mod.set_axon_ntff_profile_hook(
            _ntff_profile_via_ctypes("/opt/axon/libaxon_pjrt.so")
        )
    except Exception:
        pass


def run(inputs, trace=False):
    """Run the SPMD kernel; returns (output [4,1024,512] f32, BassKernelResults)."""
    if trace:
        _ensure_ntff_hook()
    from concourse.bass_utils import run_bass_kernel_spmd

    nc = _get_nc()
    in_maps = _make_in_maps(inputs)
    res = run_bass_kernel_spmd(nc, in_maps, core_ids=list(range(8)), trace=trace)
    outs = []
    for b in range(4):
        part = res.results[2 * b]["out"] + res.results[2 * b + 1]["out"]
        outs.append(np.ascontiguousarray(part.T))
    return np.stack(outs).astype(np.float32), res


def kernel(**inputs) -> np.ndarray:
    out, _ = run(inputs, trace=False)
    return out
